# revision 33
# baseline (speedup 1.0000x reference)
"""Bipartite graph attention layer on 8 Trainium2 NeuronCores.

Sharding: data-parallel over (batch b, n_src half). Core c handles
b = c // 2, rows n0 = (c % 2) * 1024 .. +1024 of feat_src; params and
feat_dst[b] replicated per b-pair.

Math (per batch b, head h):
  h_src = feat_src @ W[h]; h_dst = feat_dst @ W[h]
  s[n] = tanh(h_src[n]) . w_src[h];  d[m] = tanh(h_dst[m]) . w_dst[h]
  E[m, n] = exp(leaky_relu(s[n] + d[m], 0.2))
  feat_out[n] = (sum_m E[m,n] h_dst[m]) / (sum_m E[m,n]) + b

Key identity used to avoid materializing logits:
  leaky(x) = 0.2 x + 0.8 relu(x)  =>
  E = exp(0.2 s) * exp(0.2 d) * max(exp(0.8 s) exp(0.8 d), 1)
The per-n factor exp(0.2 s) cancels in the softmax ratio, so the kernel
accumulates E' = E / exp(0.2 s) = max(u8[n] * v1[m], v2[m]) where
u8 = exp(0.8 s), v1 = exp(d), v2 = exp(0.2 d): one tensor_scalar
(mult, max with per-partition scalars) per [128 m, 1024 n] tile.
sum_m comes from an extra ones-column appended to the matmul rhs.
sigmoid(g) is folded into the final combine as 0.5 (tanh(g/2) + 1) and
elu via max(y,0) - 1 + exp(min(y, 0)) so one ACT table set serves all.

v3 structure (HW-measured): all matmul operands bf16 (fp32 runs at 1/4
PE rate); gpsimd compute never used (10x slower on HW than modeled);
DMAs batched in 4-chunk pieces alternating the two HW queues; the
epilogue scale runs on ACT from 2-tile-wide PSUM accumulation banks;
elu+gate+combine runs per (head-pair, n-tile) 128-column block right
after the pair's epilogue so it overlaps the later heads' attention;
output tiles DMA out as their last column block completes.
"""

import sys

sys.path.insert(0, "/opt/trn_rl_repo")

import numpy as np

B, N_SRC, N_DST, IN_DIM, OUT_DIM, H = 4, 2048, 2048, 256, 64, 4
N = N_SRC // 2        # n_src rows per core
M = N_DST             # dst rows per core
NT = N // 128         # 8 n-tiles per core
MC = M // 128         # 16 m-chunks
RW = 66               # rhs width: 64 h_dst cols + ones col + pad col

_CACHE = {}
STAGE_LIMIT = 99      # debug: emit only the first K stages
ABLATE = set()        # debug: names of stages to skip when timing
EPI_ACT = False       # epilogue scale on ACT (else DVE tensor_scalar)
ACC_WIDTH = 2         # ns-tiles per attention PSUM bank
ACC_BUFS = 2          # psum bufs for attention accumulation groups
TRANS_COPY = "act"    # transpose psum->sbuf copies: dve/act/split
RHS_COPY = "dve"      # rhs_all copies (act measured +46us: keep dve)
INTERLEAVE_HDST = False  # emit h_dst matmuls between fdst transposes
BIAS_ZERO = True      # skip the +b add (b is zeros in setup_inputs)


def _build_program(loop_k=None):
    import concourse.bass as bass
    import concourse.tile as tile
    from concourse import mybir
    from concourse.masks import make_identity

    f32 = mybir.dt.float32
    bf16 = mybir.dt.bfloat16
    AF = mybir.ActivationFunctionType
    OP = mybir.AluOpType

    nc = bass.Bass()
    fsrc_d = nc.declare_dram_parameter("fsrc", [N, IN_DIM], f32, isOutput=False)
    fdst_d = nc.declare_dram_parameter("fdst", [M, IN_DIM], f32, isOutput=False)
    W_d = nc.declare_dram_parameter("W", [H, IN_DIM, OUT_DIM], f32, isOutput=False)
    b_d = nc.declare_dram_parameter("bias", [OUT_DIM], f32, isOutput=False)
    wsrc_d = nc.declare_dram_parameter("wsrc", [H, OUT_DIM], f32, isOutput=False)
    wdst_d = nc.declare_dram_parameter("wdst", [H, OUT_DIM], f32, isOutput=False)
    Hw_d = nc.declare_dram_parameter("Hw", [IN_DIM, IN_DIM], f32, isOutput=False)
    Hb_d = nc.declare_dram_parameter("Hb", [IN_DIM], f32, isOutput=False)
    out_d = nc.declare_dram_parameter("out", [N, IN_DIM], f32, isOutput=True)

    with tile.TileContext(nc) as tc:
        if loop_k is None:
            _emit(nc, tc, bass, mybir, make_identity, f32, bf16, AF, OP,
                  fsrc_d, fdst_d, W_d, b_d, wsrc_d, wdst_d, Hw_d, Hb_d, out_d)
        else:
            with tc.For_i(0, loop_k):
                _emit(nc, tc, bass, mybir, make_identity, f32, bf16, AF, OP,
                      fsrc_d, fdst_d, W_d, b_d, wsrc_d, wdst_d, Hw_d, Hb_d, out_d)

    _split_sync_waits(nc, mybir)
    return nc


def _emit(nc, tc, bass, mybir, make_identity, f32, bf16, AF, OP,
          fsrc_d, fdst_d, W_d, b_d, wsrc_d, wdst_d, Hw_d, Hb_d, out_d):
    from contextlib import ExitStack

    ctx = ExitStack()
    with ctx:
        const = ctx.enter_context(tc.tile_pool(name="const", bufs=1))
        head_p = ctx.enter_context(tc.tile_pool(name="head", bufs=2))
        ep_p = ctx.enter_context(tc.tile_pool(name="ep", bufs=4))
        fin_p = ctx.enter_context(tc.tile_pool(name="fin", bufs=3))
        ps_sm = ctx.enter_context(tc.tile_pool(name="ps_sm", bufs=2, space="PSUM"))
        ps_g = ctx.enter_context(tc.tile_pool(name="ps_g", bufs=2, space="PSUM"))
        ps_acc = ctx.enter_context(tc.tile_pool(name="ps_acc", bufs=2, space="PSUM"))

        # -------- loads: 4-chunk DMAs alternating the two HW queues ------
        # (gpsimd DMA fails walrus codegen inside For_i: "ISA wrong length")
        W_sb = const.tile([128, 2, H, OUT_DIM], f32)
        W_src_ap = W_d.rearrange("h (c p) o -> c p h o", p=128)
        for c in range(2):
            nc.scalar.dma_start(W_sb[:, c, :, :], W_src_ap[c])
        # small params first on SP (sub-us): needed for wsrc_rep/wdst_row
        wsrc_colT = const.tile([128, 2], f32)
        nc.sync.dma_start(wsrc_colT,
                          wsrc_d.rearrange("(pair hh) o -> (hh o) pair", hh=2))
        wdst_rows = const.tile([1, H, OUT_DIM], f32)
        nc.sync.dma_start(wdst_rows, wdst_d[None, :, :])
        Hb_row = const.tile([1, IN_DIM], f32)
        nc.sync.dma_start(Hb_row, Hb_d[None, :])

        fdst_sb = const.tile([128, MC, IN_DIM], f32)
        fdst_ap = fdst_d.rearrange("(r p) i -> p r i", p=128)
        for r0, q in ((0, nc.scalar), (4, nc.sync), (8, nc.scalar),
                      (12, nc.sync)):
            q.dma_start(fdst_sb[:, r0:r0 + 4, :], fdst_ap[:, r0:r0 + 4, :])
        fsrc_sb = const.tile([128, NT, IN_DIM], f32)
        fsrc_ap = fsrc_d.rearrange("(r p) i -> p r i", p=128)
        nc.sync.dma_start(fsrc_sb[:, 0:4, :], fsrc_ap[:, 0:4, :])
        nc.scalar.dma_start(fsrc_sb[:, 4:NT, :], fsrc_ap[:, 4:NT, :])
        Hw_sb = const.tile([128, 2, IN_DIM], f32)
        nc.scalar.dma_start(Hw_sb, Hw_d.rearrange("(r p) i -> p r i", p=128))
        if not BIAS_ZERO:
            b_full = const.tile([1, H, OUT_DIM], f32)
            for h in range(H):
                nc.sync.dma_start(b_full[:, h, :], b_d[None, :])

        ident = const.tile([128, 128], f32)
        make_identity(nc, ident)
        ones_col = const.tile([1, 128], bf16)
        nc.vector.memset(ones_col, 1.0)
        ones_col_f = const.tile([1, 128], f32)
        nc.vector.memset(ones_col_f, 1.0)
        ones128 = const.tile([128, 128], f32)
        nc.vector.memset(ones128, 1.0)

        # bf16 casts of matmul operands
        W_bf = const.tile([128, 2, H, OUT_DIM], bf16)
        nc.scalar.copy(W_bf.rearrange("p c h o -> p (c h o)"),
                       W_sb.rearrange("p c h o -> p (c h o)"))
        Hb_bf = const.tile([1, IN_DIM], bf16)
        nc.scalar.copy(Hb_bf, Hb_row)

        def copy_eng(mode, k):
            if mode == "split":
                return nc.vector.tensor_copy if k % 2 == 0 else nc.scalar.copy
            return {"dve": nc.vector.tensor_copy, "act": nc.scalar.copy}[mode]

        _tc = [0]

        def pe_transpose4(dst, srcs):
            # batch up to 4 [128,128] transposes into one psum bank + 1 copy
            ps = ps_sm.tile([128, 512], f32, tag="sm")
            for k, src in enumerate(srcs):
                nc.tensor.transpose(ps[:, 128 * k:128 * (k + 1)], src, ident)
            copy_eng(TRANS_COPY, _tc[0])(dst, ps[:, 0:128 * len(srcs)])
            _tc[0] += 1

        def bcast_row(dst, row_ap, width):
            # dst[128, width] sbuf <- row_ap[1, width] replicated to all rows
            ps = ps_sm.tile([128, 512], f32, tag="sm")
            nc.tensor.matmul(ps[:, 0:width], ones_col_f, row_ap, start=True,
                             stop=True)
            nc.vector.tensor_copy(dst, ps[:, 0:width])

        # ---------------- transposes (f32 in, bf16 out via copy) ---------
        fsrcT = [const.tile([128, N], bf16, name=f"fsrcT{c}") for c in range(2)]
        fdstT = [const.tile([128, M], bf16, name=f"fdstT{c}") for c in range(2)]
        HwT = [const.tile([128, IN_DIM], bf16, name=f"HwT{c}") for c in range(2)]
        rhs_all = const.tile([128, H, MC, RW], bf16)
        nc.vector.memset(rhs_all[:, :, :, 64:65], 1.0)
        nc.vector.memset(rhs_all[:, :, :, 65:66], 0.0)

        if "transp" in ABLATE:
            for c in range(2):
                nc.vector.memset(fsrcT[c], 0.01)
                nc.vector.memset(fdstT[c], 0.01)
                nc.vector.memset(HwT[c], 0.01)
        if "d0copy" in ABLATE:
            nc.vector.memset(
                rhs_all.rearrange("p h m w -> p (h m w)")[:, 0:H * MC * RW], 0.25)

        def emit_hdst(mc):
            hd = ps_sm.tile([128, 512], f32, tag="hd", bufs=2)
            for c in range(2):
                nc.tensor.matmul(
                    hd[:, 0:H * OUT_DIM],
                    fdstT[c][:, 128 * mc:128 * (mc + 1)],
                    W_bf[:, c, :, :].rearrange("p h o -> p (h o)"),
                    start=(c == 0), stop=(c == 1))
            if "d0copy" in ABLATE:
                return
            copy_eng(RHS_COPY, mc)(
                rhs_all[:, :, mc, 0:OUT_DIM],
                hd[:, 0:H * OUT_DIM].rearrange("p (h o) -> p h o", h=H))

        if "transp" not in ABLATE:
            for r0 in range(0, MC, 4):
                for c in range(2):
                    pe_transpose4(
                        fdstT[c][:, 128 * r0:128 * (r0 + 4)],
                        [fdst_sb[:, r, 128 * c:128 * (c + 1)]
                         for r in range(r0, r0 + 4)])
                if "d0" not in ABLATE and INTERLEAVE_HDST:
                    for mc in range(r0, r0 + 4):
                        emit_hdst(mc)
            if "d0" not in ABLATE and not INTERLEAVE_HDST:
                for mc in range(MC):
                    emit_hdst(mc)
            for c in range(2):
                for r0 in range(0, NT, 4):
                    pe_transpose4(
                        fsrcT[c][:, 128 * r0:128 * (r0 + 4)],
                        [fsrc_sb[:, r, 128 * c:128 * (c + 1)]
                         for r in range(r0, r0 + 4)])
            for c in range(2):
                pe_transpose4(
                    HwT[c][:, 0:256],
                    [Hw_sb[:, r, 128 * c:128 * (c + 1)] for r in range(2)])
        elif "d0" not in ABLATE:
            for mc in range(MC):
                emit_hdst(mc)

        # replicated w_src: wsrc_rep[64*hh + o, pair, c] = w_src[2*pair+hh, o]
        wsrc_rep = const.tile([128, 2, 128], bf16)
        for pair in range(2):
            nc.vector.tensor_scalar(wsrc_rep[:, pair, :], ones128,
                                    wsrc_colT[:, pair:pair + 1], None, OP.mult)
        # w_dst broadcast rows: wdst_row[128, h, o] = w_dst[h, o]
        wdst_row = const.tile([128, H, OUT_DIM], bf16)
        bcast_row(wdst_row.rearrange("p h o -> p (h o)"),
                  wdst_rows.rearrange("p h o -> p (h o)"), H * OUT_DIM)
        if not BIAS_ZERO:
            b_full_b = const.tile([128, H * OUT_DIM], f32)
            bcast_row(b_full_b, b_full.rearrange("p h o -> p (h o)"),
                      H * OUT_DIM)

        # ------- a_dst tails: tanh(h_dst).w_dst -> v1 = e^a, v2 = e^0.2a --
        v1_all = const.tile([128, H, MC], f32)
        v2_all = const.tile([128, H, MC], f32)
        if ABLATE & {"d0", "d0tail"}:
            nc.vector.memset(v1_all.rearrange("p h m -> p (h m)"), 1.0)
            nc.vector.memset(v2_all.rearrange("p h m -> p (h m)"), 1.0)
        else:
            for h in range(H):
                th = head_p.tile([128, MC, OUT_DIM], bf16, tag="thall")
                nc.scalar.activation(th, rhs_all[:, h, :, 0:OUT_DIM], AF.Tanh)
                tw = head_p.tile([128, MC, OUT_DIM], bf16, tag="tw")
                nc.vector.tensor_mul(
                    tw, th, wdst_row[:, h:h + 1, :].broadcast_to(
                        [128, MC, OUT_DIM]))
                a_dst = head_p.tile([128, MC], f32, tag="adst")
                nc.vector.tensor_reduce(a_dst, tw, mybir.AxisListType.X,
                                        OP.add)
                nc.scalar.activation(v1_all[:, h, :], a_dst, AF.Exp)
                nc.scalar.activation(v2_all[:, h, :], a_dst, AF.Exp, scale=0.2)

        if STAGE_LIMIT <= 1:
            return
        # bf16 copies of feat_src for the final combine: x and x+1
        xbf = const.tile([128, NT, IN_DIM], bf16)
        xp1 = const.tile([128, NT, IN_DIM], bf16)
        for half in range(2):
            sl = slice(NT // 2 * half, NT // 2 * (half + 1))
            src = fsrc_sb[:, sl, :].rearrange("p t i -> p (t i)")
            nc.scalar.copy(xbf[:, sl, :].rearrange("p t i -> p (t i)"), src)
            nc.scalar.activation(xp1[:, sl, :].rearrange("p t i -> p (t i)"),
                                 src, AF.Copy, bias=1.0)

        # ---------------- gate matmuls (tanh(g/2); sigmoid folded) -------
        tg_all = const.tile([128, NT, IN_DIM], bf16)
        for t in range(NT):
            g = ps_g.tile([128, 512], f32, tag="gsb", bufs=2)
            if "gatemm" in ABLATE:
                nc.tensor.matmul(g[:, 0:IN_DIM], ones_col, Hb_bf, start=True,
                                 stop=True)
            else:
                for c in range(2):
                    nc.tensor.matmul(g[:, 0:IN_DIM],
                                     fsrcT[c][:, 128 * t:128 * (t + 1)],
                                     HwT[c], start=(c == 0), stop=False)
                nc.tensor.matmul(g[:, 0:IN_DIM], ones_col, Hb_bf, start=False,
                                 stop=True)
            nc.scalar.activation(tg_all[:, t, :], g[:, 0:IN_DIM], AF.Tanh,
                                 scale=0.5)

        if STAGE_LIMIT <= 2:
            return
        # ---------------- heads: attention + fused epilogue/final --------
        NSG = NT // ACC_WIDTH
        feat_pre = const.tile([128, NT, H * OUT_DIM], bf16)
        out_sb = const.tile([128, NT, IN_DIM], f32)
        if "epi" in ABLATE:
            nc.vector.memset(feat_pre.rearrange("p t f -> p (t f)"), 0.1)

        def emit_final(ns):
            # elu + gate + residual for all 256 cols of tile ns (bf16 chain)
            fp = feat_pre[:, ns, :]
            if not BIAS_ZERO:
                fpb = fin_p.tile([128, IN_DIM], bf16, tag="fpb")
                nc.vector.tensor_add(fpb, fp, b_full_b)
                fp = fpb
            mn = fin_p.tile([128, IN_DIM], bf16, tag="mn")
            nc.vector.tensor_scalar(mn, fp, 0.0, None, OP.min)
            e = fin_p.tile([128, IN_DIM], bf16, tag="e")
            nc.scalar.activation(e, mn, AF.Exp)
            z2 = fin_p.tile([128, IN_DIM], bf16, tag="z2")
            nc.vector.scalar_tensor_tensor(z2, fp, 0.0, e, OP.max, OP.add)
            # z2 = max(fp, 0) + e^min(fp, 0) = elu(fp) + 1
            d = fin_p.tile([128, IN_DIM], bf16, tag="d")
            nc.vector.tensor_sub(d, z2, xp1[:, ns, :])   # d = elu - x
            m2 = fin_p.tile([128, IN_DIM], bf16, tag="m2")
            nc.vector.scalar_tensor_tensor(m2, tg_all[:, ns, :], 1.0, d,
                                           OP.add, OP.mult)
            # out = 0.5 (tg+1) (elu-x) + x = sig(g) elu + (1-sig(g)) x
            nc.vector.scalar_tensor_tensor(out_sb[:, ns, :], m2, 0.5,
                                           xbf[:, ns, :], OP.mult, OP.add)

        for pair in range(2):
            # h_srcT for head pair: psum [128 (2h, o), 512] x2 blocks
            th_srcT = head_p.tile([128, N], bf16, tag="thsrc")
            for nb in range(2 if "usrc" not in ABLATE else 0):
                hs = ps_sm.tile([128, 512], f32, tag="sm")
                for c in range(2):
                    nc.tensor.matmul(
                        hs,
                        W_bf[:, c, 2 * pair:2 * pair + 2, :].rearrange(
                            "p h o -> p (h o)"),
                        fsrcT[c][:, 512 * nb:512 * (nb + 1)],
                        start=(c == 0), stop=(c == 1))
                nc.scalar.activation(th_srcT[:, 512 * nb:512 * (nb + 1)], hs,
                                     AF.Tanh)
            for hh in range(2):
                h = 2 * pair + hh
                u8 = head_p.tile([128, N], bf16, tag="u8")
                if "usrc" in ABLATE:
                    nc.vector.memset(u8, 1.0)
                for nb in range(2 if "usrc" not in ABLATE else 0):
                    sb = ps_g.tile([128, 512], f32, tag="gsb", bufs=2)
                    nc.tensor.matmul(
                        sb, wsrc_rep[64 * hh:64 * (hh + 1), pair, :],
                        th_srcT[64 * hh:64 * (hh + 1), 512 * nb:512 * (nb + 1)],
                        start=True, stop=True)
                    nc.scalar.activation(u8[:, 512 * nb:512 * (nb + 1)], sb,
                                         AF.Exp, scale=0.8)
                Ep_all = ep_p.tile([128, MC, N], bf16, tag="Ep", bufs=2)
                for mc in range(MC if "epts" not in ABLATE else 1):
                    nc.vector.tensor_scalar(Ep_all[:, mc, :], u8,
                                            v1_all[:, h, mc:mc + 1],
                                            v2_all[:, h, mc:mc + 1],
                                            OP.mult, OP.max)
                for nsg in range(NSG):
                    acc = ps_acc.tile([128, ACC_WIDTH * RW], f32, tag="acc",
                                      bufs=ACC_BUFS)
                    for k in range(ACC_WIDTH):
                        ns = ACC_WIDTH * nsg + k
                        for mc in range(MC if "attnmm" not in ABLATE else 1):
                            nc.tensor.matmul(
                                acc[:, RW * k:RW * k + RW],
                                Ep_all[:, mc, 128 * ns:128 * (ns + 1)],
                                rhs_all[:, h, mc, :],
                                start=(mc == 0),
                                stop=(mc == MC - 1 or "attnmm" in ABLATE))
                    if "epi" in ABLATE:
                        continue
                    acc_r = acc.rearrange("p (k w) -> p k w", w=RW)
                    rec = ep_p.tile([128, ACC_WIDTH], f32, tag="rec", bufs=4)
                    nc.vector.reciprocal(rec, acc_r[:, :, 64])
                    for k in range(ACC_WIDTH):
                        ns = ACC_WIDTH * nsg + k
                        if EPI_ACT:
                            nc.scalar.activation(
                                feat_pre[:, ns, 64 * h:64 * (h + 1)],
                                acc_r[:, k, 0:OUT_DIM], AF.Copy,
                                scale=rec[:, k:k + 1])
                        else:
                            nc.vector.tensor_scalar(
                                feat_pre[:, ns, 64 * h:64 * (h + 1)],
                                acc_r[:, k, 0:OUT_DIM], rec[:, k:k + 1],
                                None, OP.mult)
                        if (h == H - 1 and STAGE_LIMIT > 3
                                and "final" not in ABLATE):
                            emit_final(ns)
                            nc.sync.dma_start(
                                out_d[128 * ns:128 * (ns + 1), :],
                                out_sb[:, ns, :])
        if "final" in ABLATE:
            for ns in range(NT):
                nc.sync.dma_start(out_d[128 * ns:128 * (ns + 1), :],
                                  out_sb[:, ns, :])


def _split_sync_waits(nc, mybir, max_waits=1, drain_max_waits=0):
    """Walrus for cayman here accepts at most one sem-wait per
    instruction (and none on Drain): move overflow waits onto preceding
    same-engine NOPs."""
    n_split = 0
    for f in nc.m.functions:
        for bb in f.blocks:
            il = bb.instructions
            i = 0
            while i < len(il):
                ins = il[i]
                si = ins.sync_info
                limit = (drain_max_waits
                         if type(ins).__name__ == "InstDrain" else max_waits)
                if si is not None and len(si.on_wait) > limit:
                    waits = list(si.on_wait)
                    keep = waits[-limit:] if limit > 0 else []
                    overflow = waits[:len(waits) - limit]
                    chunks = [overflow[j:j + max_waits]
                              for j in range(0, len(overflow), max_waits)]
                    pos = i
                    for chunk in chunks:
                        nop = mybir.InstNoOp(
                            name=f"I-waitsplit-{n_split}",
                            engine=ins.engine,
                            sync_info=mybir.SyncInfo(on_wait=chunk, on_update=[]),
                        )
                        n_split += 1
                        il.insert(pos, nop)
                        pos += 1
                        i += 1
                    ins.sync_info = mybir.SyncInfo(
                        on_wait=keep, on_update=list(si.on_update))
                i += 1
    return n_split


def _get_runner(loop_k=None):
    key = ("runner", loop_k)
    if key in _CACHE:
        return _CACHE[key]
    import jax
    from jax.sharding import Mesh, PartitionSpec
    from jax.experimental.shard_map import shard_map
    import concourse.mybir as mybir
    from concourse.bass2jax import (_bass_exec_p, install_neuronx_cc_hook,
                                    partition_id_tensor)

    nc = _build_program(loop_k)
    install_neuronx_cc_hook()
    n_cores = 8

    in_names, out_names, out_avals = [], [], []
    for alloc in nc.m.functions[0].allocations:
        if not isinstance(alloc, mybir.MemoryLocationSet):
            continue
        name = alloc.memorylocations[0].name
        if alloc.kind == "ExternalInput":
            if (nc.partition_id_tensor is not None
                    and name == nc.partition_id_tensor.name):
                continue
            in_names.append(name)
        elif alloc.kind == "ExternalOutput":
            out_names.append(name)
            out_avals.append(jax.core.ShapedArray(
                tuple(alloc.tensor_shape), mybir.dt.np(alloc.dtype)))
    n_params = len(in_names)
    in_names_all = list(in_names) + list(out_names)
    if nc.partition_id_tensor is not None:
        in_names_all.append(nc.partition_id_tensor.name)

    def _body(*args):
        operands = list(args)
        if nc.partition_id_tensor is not None:
            operands.append(partition_id_tensor())
        return tuple(_bass_exec_p.bind(
            *operands,
            out_avals=tuple(out_avals),
            in_names=tuple(in_names_all),
            out_names=tuple(out_names),
            lowering_input_output_aliases=(),
            sim_require_finite=True,
            sim_require_nnan=True,
            nc=nc,
        ))

    devices = jax.devices()[:n_cores]
    mesh = Mesh(np.asarray(devices), ("core",))
    n_outs = len(out_names)
    sharded = jax.jit(
        shard_map(_body, mesh=mesh,
                  in_specs=(PartitionSpec("core"),) * (n_params + n_outs),
                  out_specs=(PartitionSpec("core"),) * n_outs,
                  check_rep=False),
        keep_unused=True,
    )
    runner = (sharded, in_names, out_names, out_avals)
    _CACHE[key] = runner
    return runner


def _shard_inputs(feat_src, feat_dst, W, b, w_src, w_dst, H_w, H_b):
    per_core = []
    for c in range(8):
        bb, half = c // 2, c % 2
        per_core.append({
            "fsrc": np.ascontiguousarray(feat_src[bb, N * half:N * (half + 1)]),
            "fdst": np.ascontiguousarray(feat_dst[bb]),
            "W": W, "bias": b, "wsrc": w_src, "wdst": w_dst,
            "Hw": H_w, "Hb": H_b,
        })
    return per_core


def kernel(feat_src, feat_dst, W, b, w_src, w_dst, H_w, H_b):
    global BIAS_ZERO
    feat_src = np.asarray(feat_src, np.float32)
    feat_dst = np.asarray(feat_dst, np.float32)
    args = [np.asarray(a, np.float32) for a in (W, b, w_src, w_dst, H_w, H_b)]
    if not np.all(args[1] == 0.0) and BIAS_ZERO:
        BIAS_ZERO = False       # rebuild with the bias path emitted
        _CACHE.clear()
    sharded, in_names, out_names, out_avals = _get_runner()
    per_core = _shard_inputs(feat_src, feat_dst, *args)
    concat_in = [np.concatenate([per_core[c][nm] for c in range(8)], axis=0)
                 for nm in in_names]
    concat_zeros = [np.zeros((8 * av.shape[0], *av.shape[1:]), av.dtype)
                    for av in out_avals]
    outs = sharded(*concat_in, *concat_zeros)
    o = np.asarray(outs[out_names.index("out")]).reshape(8, N, IN_DIM)
    full = np.empty((B, N_SRC, IN_DIM), np.float32)
    for c in range(8):
        bb, half = c // 2, c % 2
        full[bb, N * half:N * (half + 1)] = o[c]
    return full
